# revision 1
# baseline (speedup 1.0000x reference)
"""GCN encoder (2-layer, mu/logstd heads) on 8 Trainium2 NeuronCores.

Strategy (1D graph partitioning, dst-partitioned edges):
  - Host: add self-loops, compute symmetric GCN normalization (deg^-1/2) in
    f64, build a load-balancing node permutation (round-robin deal by degree
    into blocks of 128 lanes spread over 8 cores), and lay out per-core edge
    metadata: int16 gather indices (wrapped-16 SWDGE layout, lo/hi table
    split for the int16 range), per-edge dst lane + scaled edge weight.
  - Device (single SPMD program, TileContext):
      Phase A: every core projects the full x @ W1 (stationary x^T tiles,
               moving W1), scales rows by deg^-1/2, writes a node-major bf16
               gather table to its DRAM.
      Phase B: per window of blocks: dma_gather edge source rows ->
               edge-major SBUF tiles; build scaled one-hot matrices
               omega[e,n] = (iota==dst_lane)*ew with one dual-op
               tensor_scalar per tile; PE matmuls accumulate
               sum_e omega[e,n]*msg[e,f] per 128-node block in PSUM
               (= the segment_sum); bias via a K=1 matmul; ReLU flush;
               PE-transpose h1 into a feature-major slab.
      Phase C: project own h1 shard with [Wmu||Wls], scale by deg^-1/2,
               AllGather the bf16 table2 shards across the 8 cores.
      Phase D: same message passing as B against table2, fused mu||logstd
               (64+64 columns), f32 outputs.
  - Host: inverse-permute rows, return (mu, logstd).
"""

import os
import sys

sys.path.insert(0, "/opt/trn_rl_repo")

import numpy as np
import ml_dtypes
from contextlib import ExitStack

import concourse.bass as bass
import concourse.bacc as bacc
import concourse.mybir as mybir
import concourse.tile as tile
from concourse.bass_utils import run_bass_kernel_spmd

P = 128
NCORES = 8
VLO = 32768          # int16 index range per gather table view
WINDOW_BLOCKS = int(os.environ.get("KERNEL_WB", "4"))

BF16 = mybir.dt.bfloat16
F32 = mybir.dt.float32
I16 = mybir.dt.int16
NPBF16 = ml_dtypes.bfloat16


def _ceil_div(a, b):
    return -(-a // b)


# ----------------------------------------------------------------------------
# Host preprocessing
# ----------------------------------------------------------------------------

def _build_pass_layout(src_rows, e_core, e_brow, e_lane, e_ew, nblk, n_table_rows):
    """Lay out one message-passing pass: slot every edge into
    (core, block, class, tile, partition), produce wrapped-16 int16 index
    slabs and per-slot dst-lane / edge-weight metadata."""
    n_edges = len(src_rows)
    is_lo = src_rows < VLO
    gid = (e_core * nblk + e_brow) * 2 + (~is_lo).astype(np.int64)
    order = np.argsort(gid, kind="stable")
    gid_s = gid[order]
    counts = np.bincount(gid_s, minlength=NCORES * nblk * 2)
    starts = np.concatenate([[0], np.cumsum(counts)[:-1]])
    rank = np.arange(n_edges) - starts[gid_s]

    cnt_lo = counts[0::2].reshape(NCORES, nblk)
    cnt_hi = counts[1::2].reshape(NCORES, nblk)
    K_LO = max(1, int(_ceil_div(cnt_lo.max(), P)))
    K_HI = int(_ceil_div(cnt_hi.max(), P)) if cnt_hi.max() > 0 else 0
    K = K_LO + K_HI

    windows = []
    b = 0
    while b < nblk:
        wb = min(WINDOW_BLOCKS, nblk - b)
        windows.append((b, wb))
        b += wb

    # global tile index: window w holds [lo tiles of its wb blocks][hi tiles]
    tile_base = np.zeros(nblk, np.int64)
    win_of_brow = np.zeros(nblk, np.int64)
    j_of_brow = np.zeros(nblk, np.int64)
    wb_of_brow = np.zeros(nblk, np.int64)
    base = 0
    for w, (b0, wb) in enumerate(windows):
        for j in range(wb):
            tile_base[b0 + j] = base
            win_of_brow[b0 + j] = w
            j_of_brow[b0 + j] = j
            wb_of_brow[b0 + j] = wb
        base += wb * K
    TOT_TILES = base

    e_core_s = e_core[order]
    e_brow_s = e_brow[order]
    e_lane_s = e_lane[order]
    e_ew_s = e_ew[order]
    src_s = src_rows[order]
    is_lo_s = is_lo[order]

    k_local = rank // P
    p_slot = rank % P
    wb_s = wb_of_brow[e_brow_s]
    j_s = j_of_brow[e_brow_s]
    t_in_w = np.where(is_lo_s, j_s * K_LO + k_local,
                      wb_s * K_LO + j_s * K_HI + k_local)
    gt = tile_base[e_brow_s] + t_in_w

    dst_slab = np.full((NCORES, P, TOT_TILES), -1.0, np.float32)
    ew_slab = np.zeros((NCORES, P, TOT_TILES), np.float32)
    idx32_slab = np.zeros((NCORES, P, TOT_TILES), np.int32)
    dst_slab[e_core_s, p_slot, gt] = e_lane_s.astype(np.float32)
    ew_slab[e_core_s, p_slot, gt] = e_ew_s.astype(np.float32)
    idx32_slab[e_core_s, p_slot, gt] = src_s.astype(np.int32)

    lo_cols_per_win = [wb * K_LO * P // 16 for (_, wb) in windows]
    hi_cols_per_win = [wb * K_HI * P // 16 for (_, wb) in windows]
    lo_col_base = np.concatenate([[0], np.cumsum(lo_cols_per_win)[:-1]]).astype(np.int64)
    hi_col_base = np.concatenate([[0], np.cumsum(hi_cols_per_win)[:-1]]).astype(np.int64)
    lo_idx = np.zeros((NCORES, 16, int(sum(lo_cols_per_win))), np.int16)
    hi_idx = np.zeros((NCORES, 16, max(1, int(sum(hi_cols_per_win)))), np.int16)

    flat_in_region = np.where(
        is_lo_s,
        (j_s * K_LO + k_local) * P + p_slot,
        (j_s * K_HI + k_local) * P + p_slot,
    )
    w_s = win_of_brow[e_brow_s]
    col = np.where(is_lo_s, lo_col_base[w_s], hi_col_base[w_s]) + flat_in_region // 16
    row = flat_in_region % 16
    lo_mask = is_lo_s
    lo_idx[e_core_s[lo_mask], row[lo_mask], col[lo_mask]] = src_s[lo_mask].astype(np.int16)
    if K_HI > 0:
        hi_mask = ~is_lo_s
        hi_idx[e_core_s[hi_mask], row[hi_mask], col[hi_mask]] = (
            (src_s[hi_mask] - VLO).astype(np.int16))

    return dict(
        K_LO=K_LO, K_HI=K_HI, K=K, TOT_TILES=TOT_TILES, windows=windows,
        dst_slab=dst_slab, ew_slab=ew_slab, idx32_slab=idx32_slab,
        lo_idx=np.tile(lo_idx, (1, 8, 1)), hi_idx=np.tile(hi_idx, (1, 8, 1)),
        lo_col_base=lo_col_base, hi_col_base=hi_col_base,
        n_table_rows=n_table_rows,
    )


def _preprocess(x, edge_index, weight):
    N = x.shape[0]
    s = edge_index[0].astype(np.int64)
    d = edge_index[1].astype(np.int64)
    w = weight.astype(np.float64)
    s = np.concatenate([s, np.arange(N)])
    d = np.concatenate([d, np.arange(N)])
    w = np.concatenate([w, np.ones(N)])

    deg = np.bincount(d, weights=w, minlength=N)
    dis = np.where(deg > 0, deg ** -0.5, 0.0)
    ew = w * dis[d]

    NB = NCORES * _ceil_div(_ceil_div(N, NCORES), P)
    nblk = NB // NCORES
    PAD_CORE = nblk * P
    PAD_N = NB * P

    # balance: round-robin deal nodes (sorted by degree desc) into NB blocks
    tot = np.bincount(d, minlength=N)
    order = np.argsort(-tot, kind="stable")
    blk = np.empty(N, np.int64)
    lane = np.empty(N, np.int64)
    blk[order] = np.arange(N) % NB
    lane[order] = np.arange(N) // NB
    assert lane.max() < P
    core_of = blk // nblk
    brow_of = blk % nblk
    permpos = core_of * PAD_CORE + brow_of * P + lane

    e_core = core_of[d]
    e_brow = brow_of[d]
    e_lane = lane[d]

    ROWS1 = _ceil_div(N, P) * P
    pass1 = _build_pass_layout(s, e_core, e_brow, e_lane, ew, nblk, ROWS1)
    pass2 = _build_pass_layout(permpos[s], e_core, e_brow, e_lane, ew, nblk, PAD_N)

    dis_nat = np.zeros((P, ROWS1 // P), np.float32)
    nn = np.arange(N)
    dis_nat[nn % P, nn // P] = dis.astype(np.float32)
    dis_perm = np.zeros((NCORES, P, nblk), np.float32)
    dis_perm[core_of, lane, brow_of] = dis.astype(np.float32)

    return dict(
        N=N, NB=NB, nblk=nblk, PAD_CORE=PAD_CORE, PAD_N=PAD_N, ROWS1=ROWS1,
        permpos=permpos, dis_nat=dis_nat, dis_perm=dis_perm,
        pass1=pass1, pass2=pass2,
    )


# ----------------------------------------------------------------------------
# Device program
# ----------------------------------------------------------------------------

def _emit_pass(nc, pools, pl, table_dram, idx_lo_s, idx_hi_s,
               dst_s, ew_s, iota_s, ones_s, bias_s, flush_fn, ix32_s=None):
    abl = os.environ.get("KERNEL_ABL", "")
    gmode = os.environ.get("KERNEL_GMODE", "swdge")
    const_pool = pools.get("const")
    K_LO, K_HI, K = pl["K_LO"], pl["K_HI"], pl["K"]
    windows = pl["windows"]
    lo_col_base, hi_col_base = pl["lo_col_base"], pl["hi_col_base"]
    rows = pl["n_table_rows"]
    msg_pool, omega_pool, psum_pool = pools["msg"], pools["omega"], pools["psum"]

    max_wb = max(wb for _, wb in windows)
    tbl_lo = table_dram[0:min(VLO, rows), :]
    tbl_hi = table_dram[VLO:rows, :] if rows > VLO else None

    msg_c = omega_c = None
    hyb = (gmode == "hyb") and K_HI > 0
    if "nogather" in abl:
        msg_c = const_pool.tile([P, max_wb * K, P], BF16, tag="msgc")
        nc.vector.memset(msg_c[:], 0.25)
    if "noomega" in abl:
        omega_c = const_pool.tile([P, max_wb * K * P], BF16, tag="omegac")
        nc.vector.memset(omega_c[:], 0.125)
    msg_hi_pool = pools.get("msg_hi")
    for w, (b0, wb) in enumerate(windows):
        wtiles = wb * K
        nlo_tiles = wb * K_LO
        if msg_c is not None:
            msg = msg_c
            msg_hi = msg_c
            hi_off = nlo_tiles
        elif hyb:
            msg = msg_pool.tile([P, max_wb * K_LO, P], BF16, tag="msg")
            msg_hi = msg_hi_pool.tile([P, max_wb * K_HI, P], BF16, tag="msgh")
            hi_off = 0
        else:
            msg = msg_pool.tile([P, max_wb * K, P], BF16, tag="msg")
            msg_hi = msg
            hi_off = nlo_tiles
        omega = omega_c if omega_c is not None else omega_pool.tile([P, max_wb * K * P], BF16, tag="omega")
        n_lo = wb * K_LO * P
        if "nogather" in abl:
            pass
        elif gmode == "ind":
            for t in range(wtiles):
                nc.gpsimd.indirect_dma_start(
                    out=msg[:, t, :], out_offset=None,
                    in_=table_dram[:],
                    in_offset=bass.IndirectOffsetOnAxis(
                        ap=ix32_s[:, b0 * K + t:b0 * K + t + 1], axis=0))
        else:
            nc.gpsimd.dma_gather(
                out_ap=msg[:, 0:nlo_tiles, :],
                in_ap=tbl_lo,
                idxs_ap=idx_lo_s[:, int(lo_col_base[w]):int(lo_col_base[w]) + n_lo // 16],
                num_idxs=n_lo,
                num_idxs_reg=n_lo,
                elem_size=P,
                queue_num=(2 * w) % int(os.environ.get("KERNEL_NSWQ", "1")),
                single_packet=(n_lo <= 1024),
            )
        if gmode == "hyb" and K_HI > 0 and "nogather" not in abl:
            for j in range(wb):
                for k in range(K_HI):
                    t = wb * K_LO + j * K_HI + k
                    th = j * K_HI + k
                    nc.gpsimd.indirect_dma_start(
                        out=msg_hi[:, hi_off + th, :], out_offset=None,
                        in_=table_dram[:],
                        in_offset=bass.IndirectOffsetOnAxis(
                            ap=ix32_s[:, b0 * K + t:b0 * K + t + 1], axis=0))
        elif gmode != "ind" and K_HI > 0 and "nogather" not in abl:
            n_hi = wb * K_HI * P
            nc.gpsimd.dma_gather(
                out_ap=msg_hi[:, hi_off:hi_off + wb * K_HI, :],
                in_ap=tbl_hi,
                idxs_ap=idx_hi_s[:, int(hi_col_base[w]):int(hi_col_base[w]) + n_hi // 16],
                num_idxs=n_hi,
                num_idxs_reg=n_hi,
                elem_size=P,
                queue_num=(2 * w + 1) % int(os.environ.get("KERNEL_NSWQ", "1")),
                single_packet=(n_hi <= 1024),
            )
        gt0 = b0 * K
        for t in range(wtiles if "noomega" not in abl else 0):
            nc.vector.tensor_scalar(
                out=omega[:, t * P:(t + 1) * P],
                in0=iota_s,
                scalar1=dst_s[:, gt0 + t:gt0 + t + 1],
                scalar2=ew_s[:, gt0 + t:gt0 + t + 1],
                op0=mybir.AluOpType.is_equal,
                op1=mybir.AluOpType.mult,
            )
        for j in range(wb):
            brow = b0 + j
            acc = psum_pool.tile([P, P], F32, tag="acc", space="PSUM")
            for k in range(K_LO if "nomm" not in abl else 0):
                t = j * K_LO + k
                nc.tensor.matmul(
                    out=acc[:], lhsT=omega[:, t * P:(t + 1) * P],
                    rhs=msg[:, t, :], start=(k == 0), stop=False)
            for k in range(K_HI if "nomm" not in abl else 0):
                t = wb * K_LO + j * K_HI + k
                th = hi_off + j * K_HI + k
                nc.tensor.matmul(
                    out=acc[:], lhsT=omega[:, t * P:(t + 1) * P],
                    rhs=msg_hi[:, th, :], start=False, stop=False)
            nc.tensor.matmul(out=acc[:], lhsT=ones_s, rhs=bias_s,
                             start=("nomm" in abl), stop=True)
            flush_fn(brow, acc)


def _build_program(meta, HID, OUT):
    pl1, pl2 = meta["pass1"], meta["pass2"]
    nblk = meta["nblk"]
    ROWS1, PAD_CORE, PAD_N = meta["ROWS1"], meta["PAD_CORE"], meta["PAD_N"]
    NT1 = ROWS1 // P
    HOUT = 2 * OUT

    nq = int(os.environ.get("KERNEL_NSWQ", "1"))
    scratch = int(os.environ.get("KERNEL_SCRATCH", "16384"))
    nc = bacc.Bacc(num_swdge_queues=nq, dynamic_dma_scratch_size=scratch)
    xT_t = nc.declare_dram_parameter("xT", [P, ROWS1], BF16, isOutput=False)
    W1_t = nc.declare_dram_parameter("W1", [P, HID], BF16, isOutput=False)
    Wcat_t = nc.declare_dram_parameter("Wcat", [HID, HOUT], BF16, isOutput=False)
    b1_t = nc.declare_dram_parameter("b1", [1, HID], BF16, isOutput=False)
    bcat_t = nc.declare_dram_parameter("bcat", [1, HOUT], BF16, isOutput=False)
    disn_t = nc.declare_dram_parameter("dis_nat", [P, NT1], F32, isOutput=False)
    disp_t = nc.declare_dram_parameter("dis_perm", [P, nblk], F32, isOutput=False)
    iota_t = nc.declare_dram_parameter("iota", [P, P], BF16, isOutput=False)

    lo1_t = nc.declare_dram_parameter("lo1", [P, pl1["lo_idx"].shape[2]], I16, isOutput=False)
    hi1_t = nc.declare_dram_parameter("hi1", [P, pl1["hi_idx"].shape[2]], I16, isOutput=False)
    lo2_t = nc.declare_dram_parameter("lo2", [P, pl2["lo_idx"].shape[2]], I16, isOutput=False)
    hi2_t = nc.declare_dram_parameter("hi2", [P, pl2["hi_idx"].shape[2]], I16, isOutput=False)
    ix1_t = nc.declare_dram_parameter("ix1", [P, pl1["TOT_TILES"]], mybir.dt.int32, isOutput=False)
    ix2_t = nc.declare_dram_parameter("ix2", [P, pl2["TOT_TILES"]], mybir.dt.int32, isOutput=False)
    dst1_t = nc.declare_dram_parameter("dst1", [P, pl1["TOT_TILES"]], F32, isOutput=False)
    ew1_t = nc.declare_dram_parameter("ew1", [P, pl1["TOT_TILES"]], F32, isOutput=False)
    dst2_t = nc.declare_dram_parameter("dst2", [P, pl2["TOT_TILES"]], F32, isOutput=False)
    ew2_t = nc.declare_dram_parameter("ew2", [P, pl2["TOT_TILES"]], F32, isOutput=False)

    mu_t = nc.declare_dram_parameter("mu", [PAD_CORE, OUT], F32, isOutput=True)
    ls_t = nc.declare_dram_parameter("ls", [PAD_CORE, OUT], F32, isOutput=True)

    table1 = nc.dram_tensor("table1", [ROWS1, HID], BF16)
    ag_in = nc.dram_tensor("ag_in", [PAD_CORE, HOUT], BF16)
    table2 = nc.dram_tensor("table2", [PAD_N, HOUT], BF16, addr_space="Shared")

    with tile.TileContext(nc) as tc, ExitStack() as ctx:
        const = ctx.enter_context(tc.tile_pool(name="const", bufs=1))
        xt_pool = ctx.enter_context(tc.tile_pool(name="xt", bufs=3))
        stage_pool = ctx.enter_context(tc.tile_pool(name="stage", bufs=3))
        msg_pool = ctx.enter_context(tc.tile_pool(name="msg", bufs=2))
        msg_hi_pool = ctx.enter_context(tc.tile_pool(name="msg_hi", bufs=2))
        omega_pool = ctx.enter_context(tc.tile_pool(name="omega", bufs=2))
        psum_pool = ctx.enter_context(tc.tile_pool(name="psum", bufs=4, space="PSUM"))
        tp_pool = ctx.enter_context(tc.tile_pool(name="tpsum", bufs=2, space="PSUM"))

        def load_const(param, shape, dtype):
            s = const.tile(shape, dtype, tag=param.name)
            nc.sync.dma_start(out=s[:], in_=param[:])
            return s[:]

        W1_s = load_const(W1_t, [P, HID], BF16)
        Wcat_s = load_const(Wcat_t, [HID, HOUT], BF16)
        b1_s = load_const(b1_t, [1, HID], BF16)
        bcat_s = load_const(bcat_t, [1, HOUT], BF16)
        disn_s = load_const(disn_t, [P, NT1], F32)
        disp_s = load_const(disp_t, [P, nblk], F32)
        iota_s = load_const(iota_t, [P, P], BF16)
        lo1_s = load_const(lo1_t, [P, pl1["lo_idx"].shape[2]], I16)
        hi1_s = load_const(hi1_t, [P, pl1["hi_idx"].shape[2]], I16)
        lo2_s = load_const(lo2_t, [P, pl2["lo_idx"].shape[2]], I16)
        hi2_s = load_const(hi2_t, [P, pl2["hi_idx"].shape[2]], I16)
        ix1_s = load_const(ix1_t, [P, pl1["TOT_TILES"]], mybir.dt.int32)
        ix2_s = load_const(ix2_t, [P, pl2["TOT_TILES"]], mybir.dt.int32)
        dst1_s = load_const(dst1_t, [P, pl1["TOT_TILES"]], F32)
        ew1_s = load_const(ew1_t, [P, pl1["TOT_TILES"]], F32)
        dst2_s = load_const(dst2_t, [P, pl2["TOT_TILES"]], F32)
        ew2_s = load_const(ew2_t, [P, pl2["TOT_TILES"]], F32)

        ones_s = const.tile([1, P], BF16, tag="ones")
        nc.vector.memset(ones_s[:], 1.0)
        identity_s = const.tile([P, P], BF16, tag="identity")
        nc.vector.memset(identity_s[:], 0.0)
        nc.gpsimd.affine_select(
            out=identity_s[:], in_=identity_s[:],
            compare_op=mybir.AluOpType.not_equal, fill=1.0,
            base=0, pattern=[[-1, P]], channel_multiplier=1)

        h1T = const.tile([P, nblk * P], BF16, tag="h1T")

        # ---- Phase A ----
        XSLAB = 8
        for nt0 in range(0, NT1, XSLAB):
            nsl = min(XSLAB, NT1 - nt0)
            xsl = xt_pool.tile([P, XSLAB * P], BF16, tag="xsl")
            nc.sync.dma_start(out=xsl[:, 0:nsl * P],
                              in_=xT_t[:, nt0 * P:(nt0 + nsl) * P])
            for i in range(nsl):
                nt = nt0 + i
                pr = psum_pool.tile([P, HID], F32, tag="acc", space="PSUM")
                nc.tensor.matmul(out=pr[:], lhsT=xsl[:, i * P:(i + 1) * P],
                                 rhs=W1_s, start=True, stop=True)
                t1 = stage_pool.tile([P, HID], BF16, tag="t1")
                if nt % 2 == 0:
                    nc.scalar.activation(out=t1[:], in_=pr[:],
                                         func=mybir.ActivationFunctionType.Copy,
                                         scale=disn_s[:, nt:nt + 1])
                else:
                    nc.vector.tensor_scalar(
                        out=t1[:], in0=pr[:], scalar1=disn_s[:, nt:nt + 1],
                        scalar2=None, op0=mybir.AluOpType.mult)
                nc.scalar.dma_start(out=table1[nt * P:(nt + 1) * P, :], in_=t1[:])

        tc.strict_bb_all_engine_barrier()

        # ---- Phase B ----
        pools = dict(msg=msg_pool, msg_hi=msg_hi_pool, omega=omega_pool, psum=psum_pool, const=const)

        def flush1(brow, acc):
            h1tile = stage_pool.tile([P, HID], BF16, tag="h1tile")
            nc.scalar.activation(out=h1tile[:], in_=acc[:],
                                 func=mybir.ActivationFunctionType.Relu)
            tp = tp_pool.tile([P, P], BF16, tag="tp", space="PSUM")
            nc.tensor.transpose(out=tp[:], in_=h1tile[:], identity=identity_s)
            nc.scalar.copy(out=h1T[:, brow * P:(brow + 1) * P], in_=tp[:])

        if "noB" not in os.environ.get("KERNEL_ABL", ""):
            _emit_pass(nc, pools, pl1, table1, lo1_s, hi1_s,
                       dst1_s, ew1_s, iota_s, ones_s[:], b1_s, flush1, ix32_s=ix1_s)
        else:
            nc.vector.memset(h1T[:], 0.1)

        # ---- Phase C ----
        for brow in range(nblk):
            pr = psum_pool.tile([P, HOUT], F32, tag="acc", space="PSUM")
            nc.tensor.matmul(out=pr[:], lhsT=h1T[:, brow * P:(brow + 1) * P],
                             rhs=Wcat_s, start=True, stop=True)
            t2 = stage_pool.tile([P, HOUT], BF16, tag="t1")
            nc.scalar.activation(out=t2[:], in_=pr[:],
                                 func=mybir.ActivationFunctionType.Copy,
                                 scale=disp_s[:, brow:brow + 1])
            nc.sync.dma_start(out=ag_in[brow * P:(brow + 1) * P, :], in_=t2[:])

        if "noAG" not in os.environ.get("KERNEL_ABL", ""):
            nc.gpsimd.collective_compute(
                "AllGather", mybir.AluOpType.bypass,
                replica_groups=[list(range(NCORES))],
                ins=[ag_in[:]], outs=[table2[:]])

        tc.strict_bb_all_engine_barrier()

        # ---- Phase D ----
        def flush2(brow, acc):
            o = stage_pool.tile([P, HOUT], F32, tag="otile")
            nc.scalar.copy(out=o[:], in_=acc[:])
            nc.sync.dma_start(out=mu_t[brow * P:(brow + 1) * P, :], in_=o[:, 0:OUT])
            nc.sync.dma_start(out=ls_t[brow * P:(brow + 1) * P, :], in_=o[:, OUT:HOUT])

        if "noD" not in os.environ.get("KERNEL_ABL", ""):
            _emit_pass(nc, pools, pl2, table2, lo2_s, hi2_s,
                       dst2_s, ew2_s, iota_s, ones_s[:], bcat_s, flush2, ix32_s=ix2_s)

    nc.finalize()
    return nc


# ----------------------------------------------------------------------------
# Public entry
# ----------------------------------------------------------------------------

def _prepare(x, edge_index, weight, W1, b1, Wmu, bmu, Wls, bls):
    x = np.asarray(x)
    N, IN_CH = x.shape
    HID = np.asarray(W1).shape[1]
    OUT = np.asarray(Wmu).shape[1]
    meta = _preprocess(x, np.asarray(edge_index), np.asarray(weight))
    pl1, pl2 = meta["pass1"], meta["pass2"]

    nc = _build_program(meta, HID, OUT)

    xT = np.zeros((P, meta["ROWS1"]), np.float32)
    xT[:IN_CH, :N] = np.asarray(x, np.float32).T
    Wcat = np.concatenate([np.asarray(Wmu), np.asarray(Wls)], axis=1)
    bcat = np.concatenate([np.asarray(bmu), np.asarray(bls)])
    iota = np.tile(np.arange(P, dtype=np.float32)[None, :], (P, 1))

    common = {
        "xT": xT.astype(NPBF16),
        "W1": np.asarray(W1, np.float32).astype(NPBF16),
        "Wcat": Wcat.astype(np.float32).astype(NPBF16),
        "b1": np.asarray(b1, np.float32).astype(NPBF16)[None, :],
        "bcat": bcat.astype(np.float32).astype(NPBF16)[None, :],
        "dis_nat": meta["dis_nat"],
        "iota": iota.astype(NPBF16),
    }
    in_maps = []
    for c in range(NCORES):
        m = dict(common)
        m["dis_perm"] = meta["dis_perm"][c]
        m["lo1"] = pl1["lo_idx"][c]
        m["hi1"] = pl1["hi_idx"][c]
        m["lo2"] = pl2["lo_idx"][c]
        m["hi2"] = pl2["hi_idx"][c]
        m["ix1"] = pl1["idx32_slab"][c]
        m["ix2"] = pl2["idx32_slab"][c]
        m["dst1"] = pl1["dst_slab"][c]
        m["ew1"] = pl1["ew_slab"][c]
        m["dst2"] = pl2["dst_slab"][c]
        m["ew2"] = pl2["ew_slab"][c]
        in_maps.append(m)
    return nc, in_maps, meta


def _postprocess(results, meta):
    mu_cat = np.concatenate([results[c]["mu"] for c in range(NCORES)])
    ls_cat = np.concatenate([results[c]["ls"] for c in range(NCORES)])
    mu = mu_cat[meta["permpos"]].astype(np.float32)
    ls = ls_cat[meta["permpos"]].astype(np.float32)
    return mu, ls


def _run(x, edge_index, weight, W1, b1, Wmu, bmu, Wls, bls, trace=False):
    nc, in_maps, meta = _prepare(x, edge_index, weight, W1, b1, Wmu, bmu, Wls, bls)
    res = run_bass_kernel_spmd(nc, in_maps, list(range(NCORES)), trace=trace)
    return _postprocess(res.results, meta), res


def kernel(x, edge_index, weight, W1, b1, Wmu, bmu, Wls, bls):
    (mu, ls), _ = _run(x, edge_index, weight, W1, b1, Wmu, bmu, Wls, bls)
    return mu, ls



# revision 6
# speedup vs baseline: 1.4462x; 1.4462x over previous
"""GCN encoder (2-layer, mu/logstd heads) on 8 Trainium2 NeuronCores.

Strategy (1D graph partitioning, dst-partitioned edges):
  - Host: add self-loops, fold the full symmetric normalization
    norm = deg^-1/2[s] * w * deg^-1/2[d] into per-edge weights (f64), build a
    load-balancing node permutation (round-robin deal by degree into blocks of
    128 lanes spread over 8 cores), sort each block's edges by source row for
    HBM locality, and lay out per-core edge metadata: int16 gather indices
    (wrapped-16 SWDGE layout, lo/hi table split for the int16 range), per-edge
    dst lane + normalized edge weight.
  - Device (single SPMD program, TileContext):
      Phase A: every core projects the full x @ W1 (8-tile slabs; batched
               slab-sized writes of the bf16 node-major gather table).
      Phase B: per window of blocks: dma_gather edge source rows (split
               across SWDGE queues) -> edge-major SBUF tiles; build scaled
               one-hot matrices omega[e,n] = (iota==dst_lane)*norm with one
               dual-op tensor_scalar per tile; PE matmuls accumulate
               sum_e omega[e,n]*msg[e,f] per 128-node block in PSUM
               (= the segment_sum); bias via a K=1 matmul; ReLU flush;
               PE-transpose h1; immediately project with [Wmu||Wls]
               (phase C fused) and stage bf16 rows; every CHUNK blocks fire
               a chunked AllGather of table2 so communication overlaps the
               remaining message passing.
      Phase D: same message passing against the chunk-major table2, fused
               mu||logstd (64+64 columns), f32 outputs staged in SBUF and
               written with two strided DMAs per chunk.
  - Host: inverse-permute rows, return (mu, logstd).
"""

import os
import sys

sys.path.insert(0, "/opt/trn_rl_repo")

import numpy as np
import ml_dtypes
from contextlib import ExitStack

import concourse.bass as bass
import concourse.bacc as bacc
import concourse.mybir as mybir
import concourse.tile as tile
from concourse.bass_utils import run_bass_kernel_spmd

P = 128
NCORES = 8
VLO = 32768          # int16 index range per gather table view
WINDOW_BLOCKS = int(os.environ.get("KERNEL_WB", "4"))
NSWQ = int(os.environ.get("KERNEL_NSWQ", "4"))
LOSPLIT = int(os.environ.get("KERNEL_LOSPLIT", "3"))
AG_CHUNKS = int(os.environ.get("KERNEL_AGCH", "4"))

BF16 = mybir.dt.bfloat16
F32 = mybir.dt.float32
I16 = mybir.dt.int16
NPBF16 = ml_dtypes.bfloat16


def _ceil_div(a, b):
    return -(-a // b)


# ----------------------------------------------------------------------------
# Host preprocessing
# ----------------------------------------------------------------------------

def _build_pass_layout(src_rows, e_core, e_brow, e_lane, e_ew, nblk, n_table_rows):
    """Lay out one message-passing pass: slot every edge into
    (core, block, class, tile, partition), produce wrapped-16 int16 index
    slabs and per-slot dst-lane / edge-weight metadata. Edges within each
    (core, block, class) group are sorted by source row so each gather op's
    descriptor stream walks HBM mostly forward."""
    n_edges = len(src_rows)
    is_lo = src_rows < VLO
    gid = (e_core * nblk + e_brow) * 2 + (~is_lo).astype(np.int64)
    order = np.lexsort((src_rows, gid))
    gid_s = gid[order]
    counts = np.bincount(gid_s, minlength=NCORES * nblk * 2)
    starts = np.concatenate([[0], np.cumsum(counts)[:-1]])
    rank = np.arange(n_edges) - starts[gid_s]

    cnt_lo = counts[0::2].reshape(NCORES, nblk)
    cnt_hi = counts[1::2].reshape(NCORES, nblk)
    K_LO = max(1, int(_ceil_div(cnt_lo.max(), P)))
    K_HI = int(_ceil_div(cnt_hi.max(), P)) if cnt_hi.max() > 0 else 0
    K = K_LO + K_HI

    windows = []
    b = 0
    while b < nblk:
        wb = min(WINDOW_BLOCKS, nblk - b)
        windows.append((b, wb))
        b += wb

    # global tile index: window w holds [lo tiles of its wb blocks][hi tiles]
    tile_base = np.zeros(nblk, np.int64)
    win_of_brow = np.zeros(nblk, np.int64)
    j_of_brow = np.zeros(nblk, np.int64)
    wb_of_brow = np.zeros(nblk, np.int64)
    base = 0
    for w, (b0, wb) in enumerate(windows):
        for j in range(wb):
            tile_base[b0 + j] = base
            win_of_brow[b0 + j] = w
            j_of_brow[b0 + j] = j
            wb_of_brow[b0 + j] = wb
        base += wb * K
    TOT_TILES = base

    e_core_s = e_core[order]
    e_brow_s = e_brow[order]
    e_lane_s = e_lane[order]
    e_ew_s = e_ew[order]
    src_s = src_rows[order]
    is_lo_s = is_lo[order]

    k_local = rank // P
    p_slot = rank % P
    wb_s = wb_of_brow[e_brow_s]
    j_s = j_of_brow[e_brow_s]
    t_in_w = np.where(is_lo_s, j_s * K_LO + k_local,
                      wb_s * K_LO + j_s * K_HI + k_local)
    gt = tile_base[e_brow_s] + t_in_w

    dst_slab = np.full((NCORES, P, TOT_TILES), -1.0, np.float32)
    ew_slab = np.zeros((NCORES, P, TOT_TILES), np.float32)
    dst_slab[e_core_s, p_slot, gt] = e_lane_s.astype(np.float32)
    ew_slab[e_core_s, p_slot, gt] = e_ew_s.astype(np.float32)

    lo_cols_per_win = [wb * K_LO * P // 16 for (_, wb) in windows]
    hi_cols_per_win = [wb * K_HI * P // 16 for (_, wb) in windows]
    lo_col_base = np.concatenate([[0], np.cumsum(lo_cols_per_win)[:-1]]).astype(np.int64)
    hi_col_base = np.concatenate([[0], np.cumsum(hi_cols_per_win)[:-1]]).astype(np.int64)
    lo_idx = np.zeros((NCORES, 16, int(sum(lo_cols_per_win))), np.int16)
    hi_idx = np.zeros((NCORES, 16, max(1, int(sum(hi_cols_per_win)))), np.int16)

    flat_in_region = np.where(
        is_lo_s,
        (j_s * K_LO + k_local) * P + p_slot,
        (j_s * K_HI + k_local) * P + p_slot,
    )
    w_s = win_of_brow[e_brow_s]
    col = np.where(is_lo_s, lo_col_base[w_s], hi_col_base[w_s]) + flat_in_region // 16
    row = flat_in_region % 16
    lo_mask = is_lo_s
    lo_idx[e_core_s[lo_mask], row[lo_mask], col[lo_mask]] = src_s[lo_mask].astype(np.int16)
    if K_HI > 0:
        hi_mask = ~is_lo_s
        hi_idx[e_core_s[hi_mask], row[hi_mask], col[hi_mask]] = (
            (src_s[hi_mask] - VLO).astype(np.int16))

    return dict(
        K_LO=K_LO, K_HI=K_HI, K=K, TOT_TILES=TOT_TILES, windows=windows,
        dst_slab=dst_slab, ew_slab=ew_slab,
        lo_idx=np.tile(lo_idx, (1, 8, 1)), hi_idx=np.tile(hi_idx, (1, 8, 1)),
        lo_col_base=lo_col_base, hi_col_base=hi_col_base,
        n_table_rows=n_table_rows,
    )


def _preprocess(x, edge_index, weight):
    N = x.shape[0]
    s = edge_index[0].astype(np.int64)
    d = edge_index[1].astype(np.int64)
    w = weight.astype(np.float64)
    s = np.concatenate([s, np.arange(N)])
    d = np.concatenate([d, np.arange(N)])
    w = np.concatenate([w, np.ones(N)])

    deg = np.bincount(d, weights=w, minlength=N)
    dis = np.where(deg > 0, deg ** -0.5, 0.0)
    ew = dis[s] * w * dis[d]          # full symmetric norm folded per edge

    NB = NCORES * _ceil_div(_ceil_div(N, NCORES), P)
    nblk = NB // NCORES
    PAD_CORE = nblk * P
    PAD_N = NB * P

    # balance: round-robin deal nodes (sorted by degree desc) into NB blocks
    tot = np.bincount(d, minlength=N)
    order = np.argsort(-tot, kind="stable")
    blk = np.empty(N, np.int64)
    lane = np.empty(N, np.int64)
    blk[order] = np.arange(N) % NB
    lane[order] = np.arange(N) // NB
    assert lane.max() < P
    core_of = blk // nblk
    brow_of = blk % nblk
    permpos = core_of * PAD_CORE + brow_of * P + lane

    # chunk-major table2 layout: AllGather fires per chunk of CH block-rows,
    # each chunk's output is [core, ch*P, HOUT] at base 8*P*c0
    CH = _ceil_div(nblk, AG_CHUNKS)
    c_of = brow_of // CH
    c0_of = c_of * CH
    ch_of = np.minimum(CH, nblk - c0_of)
    table2pos = (NCORES * P * c0_of + core_of * (ch_of * P)
                 + (brow_of - c0_of) * P + lane)

    e_core = core_of[d]
    e_brow = brow_of[d]
    e_lane = lane[d]

    ROWS1 = _ceil_div(N, P) * P
    pass1 = _build_pass_layout(s, e_core, e_brow, e_lane, ew, nblk, ROWS1)
    pass2 = _build_pass_layout(table2pos[s], e_core, e_brow, e_lane, ew, nblk, PAD_N)

    return dict(
        N=N, NB=NB, nblk=nblk, PAD_CORE=PAD_CORE, PAD_N=PAD_N, ROWS1=ROWS1,
        CH=CH, permpos=permpos, pass1=pass1, pass2=pass2,
    )


# ----------------------------------------------------------------------------
# Device program
# ----------------------------------------------------------------------------

def _emit_gathers(nc, pl, w, b0, wb, table_dram, idx_lo_s, idx_hi_s, msg, msg_hi,
                  hi_off, qctr):
    """Emit this window's gather DMAs, splitting the lo region across SWDGE
    queues. Returns the updated rotating queue counter."""
    K_LO, K_HI = pl["K_LO"], pl["K_HI"]
    lo_col_base, hi_col_base = pl["lo_col_base"], pl["hi_col_base"]
    rows = pl["n_table_rows"]
    tbl_lo = table_dram[0:min(VLO, rows), :]
    tbl_hi = table_dram[VLO:rows, :] if rows > VLO else None
    nlo_tiles = wb * K_LO

    nsplit = max(1, min(LOSPLIT, nlo_tiles))
    bounds = [nlo_tiles * i // nsplit for i in range(nsplit + 1)]
    for i in range(nsplit):
        t0, t1 = bounds[i], bounds[i + 1]
        if t1 <= t0:
            continue
        n_idx = (t1 - t0) * P
        c0 = int(lo_col_base[w]) + t0 * (P // 16)
        nc.gpsimd.dma_gather(
            out_ap=msg[:, t0:t1, :],
            in_ap=tbl_lo,
            idxs_ap=idx_lo_s[:, c0:c0 + n_idx // 16],
            num_idxs=n_idx,
            num_idxs_reg=n_idx,
            elem_size=P,
            queue_num=qctr % NSWQ,
            single_packet=(n_idx <= 1024),
        )
        qctr += 1
    if K_HI > 0:
        n_hi = wb * K_HI * P
        nc.gpsimd.dma_gather(
            out_ap=msg_hi[:, hi_off:hi_off + wb * K_HI, :],
            in_ap=tbl_hi,
            idxs_ap=idx_hi_s[:, int(hi_col_base[w]):int(hi_col_base[w]) + n_hi // 16],
            num_idxs=n_hi,
            num_idxs_reg=n_hi,
            elem_size=P,
            queue_num=qctr % NSWQ,
            single_packet=(n_hi <= 1024),
        )
        qctr += 1
    return qctr


def _emit_pass(nc, pools, pl, table_dram, idx_lo_s, idx_hi_s,
               dst_s, ew_s, iota_s, ones_s, bias_s, flush_fn):
    abl = os.environ.get("KERNEL_ABL", "")
    const_pool = pools.get("const")
    K_LO, K_HI, K = pl["K_LO"], pl["K_HI"], pl["K"]
    windows = pl["windows"]
    msg_pool, omega_pool, psum_pool = pools["msg"], pools["omega"], pools["psum"]

    max_wb = max(wb for _, wb in windows)
    msg_c = omega_c = None
    if "nogather" in abl:
        msg_c = const_pool.tile([P, max_wb * K, P], BF16, tag="msgc")
        nc.vector.memset(msg_c[:], 0.25)
    if "noomega" in abl:
        omega_c = const_pool.tile([P, max_wb * K * P], BF16, tag="omegac")
        nc.vector.memset(omega_c[:], 0.125)
    qctr = 0
    for w, (b0, wb) in enumerate(windows):
        wtiles = wb * K
        nlo_tiles = wb * K_LO
        if msg_c is not None:
            msg = msg_c
        else:
            msg = msg_pool.tile([P, max_wb * K, P], BF16, tag="msg")
        hi_off = nlo_tiles
        omega = omega_c if omega_c is not None else omega_pool.tile(
            [P, max_wb * K * P], BF16, tag="omega")
        if "nogather" not in abl:
            qctr = _emit_gathers(nc, pl, w, b0, wb, table_dram, idx_lo_s,
                                 idx_hi_s, msg, msg, hi_off, qctr)
        gt0 = b0 * K
        for t in range(wtiles if "noomega" not in abl else 0):
            nc.vector.tensor_scalar(
                out=omega[:, t * P:(t + 1) * P],
                in0=iota_s,
                scalar1=dst_s[:, gt0 + t:gt0 + t + 1],
                scalar2=ew_s[:, gt0 + t:gt0 + t + 1],
                op0=mybir.AluOpType.is_equal,
                op1=mybir.AluOpType.mult,
            )
        for j in range(wb):
            brow = b0 + j
            acc = psum_pool.tile([P, P], F32, tag="acc", space="PSUM")
            for k in range(K_LO if "nomm" not in abl else 0):
                t = j * K_LO + k
                nc.tensor.matmul(
                    out=acc[:], lhsT=omega[:, t * P:(t + 1) * P],
                    rhs=msg[:, t, :], start=(k == 0), stop=False)
            for k in range(K_HI if "nomm" not in abl else 0):
                t = wb * K_LO + j * K_HI + k
                nc.tensor.matmul(
                    out=acc[:], lhsT=omega[:, t * P:(t + 1) * P],
                    rhs=msg[:, t, :], start=False, stop=False)
            nc.tensor.matmul(out=acc[:], lhsT=ones_s, rhs=bias_s,
                             start=("nomm" in abl), stop=True)
            flush_fn(brow, acc)


def _build_program(meta, HID, OUT):
    pl1, pl2 = meta["pass1"], meta["pass2"]
    nblk = meta["nblk"]
    CH = meta["CH"]
    ROWS1, PAD_CORE, PAD_N = meta["ROWS1"], meta["PAD_CORE"], meta["PAD_N"]
    NT1 = ROWS1 // P
    HOUT = 2 * OUT
    abl = os.environ.get("KERNEL_ABL", "")

    scratch = int(os.environ.get("KERNEL_SCRATCH", "16384"))
    nc = bacc.Bacc(num_swdge_queues=NSWQ, dynamic_dma_scratch_size=scratch)
    xT_t = nc.declare_dram_parameter("xT", [P, ROWS1], BF16, isOutput=False)
    W1_t = nc.declare_dram_parameter("W1", [P, HID], BF16, isOutput=False)
    Wcat_t = nc.declare_dram_parameter("Wcat", [HID, HOUT], BF16, isOutput=False)
    b1_t = nc.declare_dram_parameter("b1", [1, HID], BF16, isOutput=False)
    bcat_t = nc.declare_dram_parameter("bcat", [1, HOUT], BF16, isOutput=False)
    iota_t = nc.declare_dram_parameter("iota", [P, P], BF16, isOutput=False)

    lo1_t = nc.declare_dram_parameter("lo1", [P, pl1["lo_idx"].shape[2]], I16, isOutput=False)
    hi1_t = nc.declare_dram_parameter("hi1", [P, pl1["hi_idx"].shape[2]], I16, isOutput=False)
    lo2_t = nc.declare_dram_parameter("lo2", [P, pl2["lo_idx"].shape[2]], I16, isOutput=False)
    hi2_t = nc.declare_dram_parameter("hi2", [P, pl2["hi_idx"].shape[2]], I16, isOutput=False)
    dst1_t = nc.declare_dram_parameter("dst1", [P, pl1["TOT_TILES"]], F32, isOutput=False)
    ew1_t = nc.declare_dram_parameter("ew1", [P, pl1["TOT_TILES"]], F32, isOutput=False)
    dst2_t = nc.declare_dram_parameter("dst2", [P, pl2["TOT_TILES"]], F32, isOutput=False)
    ew2_t = nc.declare_dram_parameter("ew2", [P, pl2["TOT_TILES"]], F32, isOutput=False)

    mu_t = nc.declare_dram_parameter("mu", [PAD_CORE, OUT], F32, isOutput=True)
    ls_t = nc.declare_dram_parameter("ls", [PAD_CORE, OUT], F32, isOutput=True)

    table1 = nc.dram_tensor("table1", [ROWS1, HID], BF16)
    ag_in = nc.dram_tensor("ag_in", [PAD_CORE, HOUT], BF16)
    table2 = nc.dram_tensor("table2", [PAD_N, HOUT], BF16, addr_space="Shared")

    with tile.TileContext(nc) as tc, ExitStack() as ctx:
        const = ctx.enter_context(tc.tile_pool(name="const", bufs=1))
        xt_pool = ctx.enter_context(tc.tile_pool(name="xt", bufs=3))
        stage_pool = ctx.enter_context(tc.tile_pool(name="stage", bufs=3))
        msg_pool = ctx.enter_context(tc.tile_pool(name="msg", bufs=2))
        omega_pool = ctx.enter_context(tc.tile_pool(name="omega", bufs=2))
        psum_pool = ctx.enter_context(tc.tile_pool(name="psum", bufs=4, space="PSUM"))
        tp_pool = ctx.enter_context(tc.tile_pool(name="tpsum", bufs=2, space="PSUM"))

        def load_const(param, shape, dtype):
            s = const.tile(shape, dtype, tag=param.name)
            nc.sync.dma_start(out=s[:], in_=param[:])
            return s[:]

        W1_s = load_const(W1_t, [P, HID], BF16)
        Wcat_s = load_const(Wcat_t, [HID, HOUT], BF16)
        b1_s = load_const(b1_t, [1, HID], BF16)
        bcat_s = load_const(bcat_t, [1, HOUT], BF16)
        iota_s = load_const(iota_t, [P, P], BF16)
        lo1_s = load_const(lo1_t, [P, pl1["lo_idx"].shape[2]], I16)
        hi1_s = load_const(hi1_t, [P, pl1["hi_idx"].shape[2]], I16)
        lo2_s = load_const(lo2_t, [P, pl2["lo_idx"].shape[2]], I16)
        hi2_s = load_const(hi2_t, [P, pl2["hi_idx"].shape[2]], I16)
        dst1_s = load_const(dst1_t, [P, pl1["TOT_TILES"]], F32)
        ew1_s = load_const(ew1_t, [P, pl1["TOT_TILES"]], F32)
        dst2_s = load_const(dst2_t, [P, pl2["TOT_TILES"]], F32)
        ew2_s = load_const(ew2_t, [P, pl2["TOT_TILES"]], F32)

        ones_s = const.tile([1, P], BF16, tag="ones")
        nc.vector.memset(ones_s[:], 1.0)
        identity_s = const.tile([P, P], BF16, tag="identity")
        nc.vector.memset(identity_s[:], 0.0)
        nc.gpsimd.affine_select(
            out=identity_s[:], in_=identity_s[:],
            compare_op=mybir.AluOpType.not_equal, fill=1.0,
            base=0, pattern=[[-1, P]], channel_multiplier=1)

        h1T = const.tile([P, nblk * P], BF16, tag="h1T")

        # ---- Phase A: full x @ W1, batched slab writes of table1 ----
        XSLAB = 8
        for nt0 in range(0, NT1, XSLAB):
            nsl = min(XSLAB, NT1 - nt0)
            xsl = xt_pool.tile([P, XSLAB * P], BF16, tag="xsl")
            nc.sync.dma_start(out=xsl[:, 0:nsl * P],
                              in_=xT_t[:, nt0 * P:(nt0 + nsl) * P])
            slab = stage_pool.tile([P, XSLAB * HID], BF16, tag="t1slab")
            for i in range(nsl):
                pr = psum_pool.tile([P, HID], F32, tag="acc", space="PSUM")
                nc.tensor.matmul(out=pr[:], lhsT=xsl[:, i * P:(i + 1) * P],
                                 rhs=W1_s, start=True, stop=True)
                if i % 2 == 0:
                    nc.scalar.copy(out=slab[:, i * HID:(i + 1) * HID], in_=pr[:])
                else:
                    nc.vector.tensor_scalar(
                        out=slab[:, i * HID:(i + 1) * HID], in0=pr[:],
                        scalar1=1.0, scalar2=None, op0=mybir.AluOpType.mult)
            nc.sync.dma_start(
                out=table1[nt0 * P:(nt0 + nsl) * P, :].rearrange(
                    "(i p) f -> p i f", p=P),
                in_=slab[:, 0:nsl * HID].rearrange("p (i f) -> p i f", f=HID))

        tc.strict_bb_all_engine_barrier()

        # ---- Phase B (+ fused C and chunked AllGather) ----
        pools = dict(msg=msg_pool, omega=omega_pool, psum=psum_pool, const=const)

        ag_state = dict(slab=None, c0=0)

        def flush_ag_chunk(c0, c1):
            ch = c1 - c0
            slab = ag_state["slab"]
            nc.sync.dma_start(
                out=ag_in[c0 * P:c1 * P, :].rearrange("(i p) f -> p i f", p=P),
                in_=slab[:, 0:ch * HOUT].rearrange("p (i f) -> p i f", f=HOUT))
            if "noAG" not in abl:
                base8 = NCORES * c0 * P
                nc.gpsimd.collective_compute(
                    "AllGather", mybir.AluOpType.bypass,
                    replica_groups=[list(range(NCORES))],
                    ins=[ag_in[c0 * P:c1 * P, :]],
                    outs=[table2[base8:base8 + NCORES * ch * P, :]])

        def flush1(brow, acc):
            h1tile = stage_pool.tile([P, HID], BF16, tag="h1tile")
            nc.scalar.activation(out=h1tile[:], in_=acc[:],
                                 func=mybir.ActivationFunctionType.Relu)
            tp = tp_pool.tile([P, P], BF16, tag="tp", space="PSUM")
            nc.tensor.transpose(out=tp[:], in_=h1tile[:], identity=identity_s)
            nc.scalar.copy(out=h1T[:, brow * P:(brow + 1) * P], in_=tp[:])
            # fused phase C: project this block and stage bf16 rows
            if brow % CH == 0:
                agslab = stage_pool.tile([P, CH * HOUT], BF16, tag="agslab")
                ag_state["slab"] = agslab
                ag_state["c0"] = brow
            pr = psum_pool.tile([P, HOUT], F32, tag="acc", space="PSUM")
            nc.tensor.matmul(out=pr[:], lhsT=h1T[:, brow * P:(brow + 1) * P],
                             rhs=Wcat_s, start=True, stop=True)
            o = brow - ag_state["c0"]
            nc.vector.tensor_scalar(
                out=ag_state["slab"][:, o * HOUT:(o + 1) * HOUT], in0=pr[:],
                scalar1=1.0, scalar2=None, op0=mybir.AluOpType.mult)
            if brow == nblk - 1 or brow % CH == CH - 1:
                flush_ag_chunk(ag_state["c0"], brow + 1)

        if "noB" not in abl:
            _emit_pass(nc, pools, pl1, table1, lo1_s, hi1_s,
                       dst1_s, ew1_s, iota_s, ones_s[:], b1_s, flush1)
        else:
            nc.vector.memset(h1T[:], 0.1)
            for c in range(0, nblk, CH):
                agslab = stage_pool.tile([P, CH * HOUT], BF16, tag="agslab")
                ag_state["slab"] = agslab
                ag_state["c0"] = c
                nc.vector.memset(ag_state["slab"][:], 0.05)
                flush_ag_chunk(c, min(nblk, c + CH))

        tc.strict_bb_all_engine_barrier()

        # ---- Phase D ----
        OCH = 13
        o_state = dict(slab=None, c0=0)

        def flush_out_chunk(c0, c1):
            ch = c1 - c0
            slab = o_state["slab"]
            sl = slab[:, 0:ch * HOUT].rearrange("p (i f) -> p i f", f=HOUT)
            nc.sync.dma_start(
                out=mu_t[c0 * P:c1 * P, :].rearrange("(i p) f -> p i f", p=P),
                in_=sl[:, :, 0:OUT])
            nc.sync.dma_start(
                out=ls_t[c0 * P:c1 * P, :].rearrange("(i p) f -> p i f", p=P),
                in_=sl[:, :, OUT:HOUT])

        def flush2(brow, acc):
            if brow % OCH == 0:
                oslab = stage_pool.tile([P, OCH * HOUT], F32, tag="oslab")
                o_state["slab"] = oslab
                o_state["c0"] = brow
            o = brow - o_state["c0"]
            if brow % 2 == 0:
                nc.scalar.copy(out=o_state["slab"][:, o * HOUT:(o + 1) * HOUT],
                               in_=acc[:])
            else:
                nc.vector.tensor_scalar(
                    out=o_state["slab"][:, o * HOUT:(o + 1) * HOUT], in0=acc[:],
                    scalar1=1.0, scalar2=None, op0=mybir.AluOpType.mult)
            if brow == nblk - 1 or brow % OCH == OCH - 1:
                flush_out_chunk(o_state["c0"], brow + 1)

        if "noD" not in abl:
            _emit_pass(nc, pools, pl2, table2, lo2_s, hi2_s,
                       dst2_s, ew2_s, iota_s, ones_s[:], bcat_s, flush2)

    nc.finalize()
    return nc


# ----------------------------------------------------------------------------
# Public entry
# ----------------------------------------------------------------------------

def _prepare(x, edge_index, weight, W1, b1, Wmu, bmu, Wls, bls):
    x = np.asarray(x)
    N, IN_CH = x.shape
    HID = np.asarray(W1).shape[1]
    OUT = np.asarray(Wmu).shape[1]
    meta = _preprocess(x, np.asarray(edge_index), np.asarray(weight))
    pl1, pl2 = meta["pass1"], meta["pass2"]

    nc = _build_program(meta, HID, OUT)

    xT = np.zeros((P, meta["ROWS1"]), np.float32)
    xT[:IN_CH, :N] = np.asarray(x, np.float32).T
    Wcat = np.concatenate([np.asarray(Wmu), np.asarray(Wls)], axis=1)
    bcat = np.concatenate([np.asarray(bmu), np.asarray(bls)])
    iota = np.tile(np.arange(P, dtype=np.float32)[None, :], (P, 1))

    common = {
        "xT": xT.astype(NPBF16),
        "W1": np.asarray(W1, np.float32).astype(NPBF16),
        "Wcat": Wcat.astype(np.float32).astype(NPBF16),
        "b1": np.asarray(b1, np.float32).astype(NPBF16)[None, :],
        "bcat": bcat.astype(np.float32).astype(NPBF16)[None, :],
        "iota": iota.astype(NPBF16),
    }
    in_maps = []
    for c in range(NCORES):
        m = dict(common)
        m["lo1"] = pl1["lo_idx"][c]
        m["hi1"] = pl1["hi_idx"][c]
        m["lo2"] = pl2["lo_idx"][c]
        m["hi2"] = pl2["hi_idx"][c]
        m["dst1"] = pl1["dst_slab"][c]
        m["ew1"] = pl1["ew_slab"][c]
        m["dst2"] = pl2["dst_slab"][c]
        m["ew2"] = pl2["ew_slab"][c]
        in_maps.append(m)
    return nc, in_maps, meta


def _postprocess(results, meta):
    mu_cat = np.concatenate([results[c]["mu"] for c in range(NCORES)])
    ls_cat = np.concatenate([results[c]["ls"] for c in range(NCORES)])
    mu = mu_cat[meta["permpos"]].astype(np.float32)
    ls = ls_cat[meta["permpos"]].astype(np.float32)
    return mu, ls


def _run(x, edge_index, weight, W1, b1, Wmu, bmu, Wls, bls, trace=False):
    nc, in_maps, meta = _prepare(x, edge_index, weight, W1, b1, Wmu, bmu, Wls, bls)
    res = run_bass_kernel_spmd(nc, in_maps, list(range(NCORES)), trace=trace)
    return _postprocess(res.results, meta), res


def kernel(x, edge_index, weight, W1, b1, Wmu, bmu, Wls, bls):
    (mu, ls), _ = _run(x, edge_index, weight, W1, b1, Wmu, bmu, Wls, bls)
    return mu, ls


# revision 17
# speedup vs baseline: 1.5867x; 1.0972x over previous
"""GCN encoder (2-layer, mu/logstd heads) on 8 Trainium2 NeuronCores.

Strategy (1D graph partitioning, dst-partitioned edges):
  - Host: add self-loops, fold the full symmetric normalization
    norm = deg^-1/2[s] * w * deg^-1/2[d] into per-edge weights (f64), build a
    load-balancing node permutation (round-robin deal by degree into blocks of
    128 lanes spread over 8 cores), sort each block's edges by source row for
    HBM locality, and lay out per-core edge metadata: int16 gather indices
    (wrapped-16 SWDGE layout, lo/hi table split for the int16 range), per-edge
    dst lane + normalized edge weight.
  - Device (single SPMD program, TileContext):
      Phase A: every core projects the full x @ W1 (8-tile slabs; batched
               slab-sized writes of the bf16 node-major gather table).
      Phase B: per window of blocks: dma_gather edge source rows (split
               across SWDGE queues) -> edge-major SBUF tiles; build scaled
               one-hot matrices omega[e,n] = (iota==dst_lane)*norm with one
               dual-op tensor_scalar per tile; PE matmuls accumulate
               sum_e omega[e,n]*msg[e,f] per 128-node block in PSUM
               (= the segment_sum); bias via a K=1 matmul; ReLU flush;
               PE-transpose h1; immediately project with [Wmu||Wls]
               (phase C fused) and stage bf16 rows; every CHUNK blocks fire
               a chunked AllGather of table2 so communication overlaps the
               remaining message passing.
      Phase D: same message passing against the chunk-major table2, fused
               mu||logstd (64+64 columns), f32 outputs staged in SBUF and
               written with two strided DMAs per chunk.
  - Host: inverse-permute rows, return (mu, logstd).
"""

import os
import sys

sys.path.insert(0, "/opt/trn_rl_repo")

import numpy as np
import ml_dtypes
from contextlib import ExitStack

import concourse.bass as bass
import concourse.bacc as bacc
import concourse.mybir as mybir
import concourse.tile as tile
from concourse.bass_utils import run_bass_kernel_spmd

P = 128
NCORES = 8
VLO = 32768          # int16 index range per gather table view
WINDOW_BLOCKS = int(os.environ.get("KERNEL_WB", "4"))
NSWQ = int(os.environ.get("KERNEL_NSWQ", "4"))
LOSPLIT = int(os.environ.get("KERNEL_LOSPLIT", "3"))
AG_CHUNKS = int(os.environ.get("KERNEL_AGCH", "8"))

BF16 = mybir.dt.bfloat16
F32 = mybir.dt.float32
I16 = mybir.dt.int16
NPBF16 = ml_dtypes.bfloat16


def _ceil_div(a, b):
    return -(-a // b)


# ----------------------------------------------------------------------------
# Host preprocessing
# ----------------------------------------------------------------------------

def _build_pass_layout(src_rows, e_core, e_brow, e_lane, e_ew, nblk, n_table_rows):
    """Lay out one message-passing pass: slot every edge into
    (core, block, class, tile, partition), produce wrapped-16 int16 index
    slabs and per-slot dst-lane / edge-weight metadata. Edges within each
    (core, block, class) group are sorted by source row so each gather op's
    descriptor stream walks HBM mostly forward."""
    n_edges = len(src_rows)
    is_lo = src_rows < VLO
    gid = (e_core * nblk + e_brow) * 2 + (~is_lo).astype(np.int64)
    order = np.lexsort((src_rows, gid))
    gid_s = gid[order]
    counts = np.bincount(gid_s, minlength=NCORES * nblk * 2)
    starts = np.concatenate([[0], np.cumsum(counts)[:-1]])
    rank = np.arange(n_edges) - starts[gid_s]

    cnt_lo = counts[0::2].reshape(NCORES, nblk)
    cnt_hi = counts[1::2].reshape(NCORES, nblk)
    K_LO = max(1, int(_ceil_div(cnt_lo.max(), P)))
    K_HI = int(_ceil_div(cnt_hi.max(), P)) if cnt_hi.max() > 0 else 0
    K = K_LO + K_HI

    windows = []
    b = 0
    while b < nblk:
        wb = min(WINDOW_BLOCKS, nblk - b)
        windows.append((b, wb))
        b += wb

    # global tile index: window w holds [lo tiles of its wb blocks][hi tiles]
    tile_base = np.zeros(nblk, np.int64)
    win_of_brow = np.zeros(nblk, np.int64)
    j_of_brow = np.zeros(nblk, np.int64)
    wb_of_brow = np.zeros(nblk, np.int64)
    base = 0
    for w, (b0, wb) in enumerate(windows):
        for j in range(wb):
            tile_base[b0 + j] = base
            win_of_brow[b0 + j] = w
            j_of_brow[b0 + j] = j
            wb_of_brow[b0 + j] = wb
        base += wb * K
    TOT_TILES = base

    e_core_s = e_core[order]
    e_brow_s = e_brow[order]
    e_lane_s = e_lane[order]
    e_ew_s = e_ew[order]
    src_s = src_rows[order]
    is_lo_s = is_lo[order]

    k_local = rank // P
    p_slot = rank % P
    wb_s = wb_of_brow[e_brow_s]
    j_s = j_of_brow[e_brow_s]
    t_in_w = np.where(is_lo_s, j_s * K_LO + k_local,
                      wb_s * K_LO + j_s * K_HI + k_local)
    gt = tile_base[e_brow_s] + t_in_w

    dst_slab = np.full((NCORES, P, TOT_TILES), -1.0, np.float32)
    ew_slab = np.zeros((NCORES, P, TOT_TILES), np.float32)
    dst_slab[e_core_s, p_slot, gt] = e_lane_s.astype(np.float32)
    ew_slab[e_core_s, p_slot, gt] = e_ew_s.astype(np.float32)

    lo_cols_per_win = [wb * K_LO * P // 16 for (_, wb) in windows]
    hi_cols_per_win = [wb * K_HI * P // 16 for (_, wb) in windows]
    lo_col_base = np.concatenate([[0], np.cumsum(lo_cols_per_win)[:-1]]).astype(np.int64)
    hi_col_base = np.concatenate([[0], np.cumsum(hi_cols_per_win)[:-1]]).astype(np.int64)
    lo_idx = np.zeros((NCORES, 16, int(sum(lo_cols_per_win))), np.int16)
    hi_idx = np.zeros((NCORES, 16, max(1, int(sum(hi_cols_per_win)))), np.int16)

    flat_in_region = np.where(
        is_lo_s,
        (j_s * K_LO + k_local) * P + p_slot,
        (j_s * K_HI + k_local) * P + p_slot,
    )
    w_s = win_of_brow[e_brow_s]
    col = np.where(is_lo_s, lo_col_base[w_s], hi_col_base[w_s]) + flat_in_region // 16
    row = flat_in_region % 16
    lo_mask = is_lo_s
    lo_idx[e_core_s[lo_mask], row[lo_mask], col[lo_mask]] = src_s[lo_mask].astype(np.int16)
    if K_HI > 0:
        hi_mask = ~is_lo_s
        hi_idx[e_core_s[hi_mask], row[hi_mask], col[hi_mask]] = (
            (src_s[hi_mask] - VLO).astype(np.int16))

    return dict(
        K_LO=K_LO, K_HI=K_HI, K=K, TOT_TILES=TOT_TILES, windows=windows,
        dst_slab=dst_slab, ew_slab=ew_slab,
        lo_idx=np.tile(lo_idx, (1, 8, 1)), hi_idx=np.tile(hi_idx, (1, 8, 1)),
        lo_col_base=lo_col_base, hi_col_base=hi_col_base,
        n_table_rows=n_table_rows,
    )


def _preprocess(x, edge_index, weight):
    N = x.shape[0]
    s = edge_index[0].astype(np.int64)
    d = edge_index[1].astype(np.int64)
    w = weight.astype(np.float64)
    s = np.concatenate([s, np.arange(N)])
    d = np.concatenate([d, np.arange(N)])
    w = np.concatenate([w, np.ones(N)])

    deg = np.bincount(d, weights=w, minlength=N)
    dis = np.where(deg > 0, deg ** -0.5, 0.0)
    ew = dis[s] * w * dis[d]          # full symmetric norm folded per edge

    NB = NCORES * _ceil_div(_ceil_div(N, NCORES), P)
    nblk = NB // NCORES
    PAD_CORE = nblk * P
    PAD_N = NB * P

    # balance: round-robin deal nodes (sorted by degree desc) into NB blocks
    tot = np.bincount(d, minlength=N)
    order = np.argsort(-tot, kind="stable")
    blk = np.empty(N, np.int64)
    lane = np.empty(N, np.int64)
    blk[order] = np.arange(N) % NB
    lane[order] = np.arange(N) // NB
    assert lane.max() < P
    core_of = blk // nblk
    brow_of = blk % nblk
    permpos = core_of * PAD_CORE + brow_of * P + lane

    # chunk-major table2 layout: AllGather fires per chunk of CH block-rows,
    # each chunk's output is [core, ch*P, HOUT] at base 8*P*c0
    CH = _ceil_div(nblk, AG_CHUNKS)
    c_of = brow_of // CH
    c0_of = c_of * CH
    ch_of = np.minimum(CH, nblk - c0_of)
    table2pos = (NCORES * P * c0_of + core_of * (ch_of * P)
                 + (brow_of - c0_of) * P + lane)

    e_core = core_of[d]
    e_brow = brow_of[d]
    e_lane = lane[d]

    ROWS1 = _ceil_div(N, P) * P
    pass1 = _build_pass_layout(s, e_core, e_brow, e_lane, ew, nblk, ROWS1)
    pass2 = _build_pass_layout(table2pos[s], e_core, e_brow, e_lane, ew, nblk, PAD_N)

    return dict(
        N=N, NB=NB, nblk=nblk, PAD_CORE=PAD_CORE, PAD_N=PAD_N, ROWS1=ROWS1,
        CH=CH, permpos=permpos, pass1=pass1, pass2=pass2,
    )


# ----------------------------------------------------------------------------
# Device program
# ----------------------------------------------------------------------------

def _emit_gathers(nc, pl, w, b0, wb, table_dram, idx_lo_s, idx_hi_s, msg, msg_hi,
                  hi_off, qctr):
    """Emit this window's gather DMAs, splitting the lo region across SWDGE
    queues. Returns the updated rotating queue counter."""
    K_LO, K_HI = pl["K_LO"], pl["K_HI"]
    lo_col_base, hi_col_base = pl["lo_col_base"], pl["hi_col_base"]
    rows = pl["n_table_rows"]
    tbl_lo = table_dram[0:min(VLO, rows), :]
    tbl_hi = table_dram[VLO:rows, :] if rows > VLO else None
    nlo_tiles = wb * K_LO

    nsplit = max(1, min(LOSPLIT, nlo_tiles))
    bounds = [nlo_tiles * i // nsplit for i in range(nsplit + 1)]
    for i in range(nsplit):
        t0, t1 = bounds[i], bounds[i + 1]
        if t1 <= t0:
            continue
        n_idx = (t1 - t0) * P
        c0 = int(lo_col_base[w]) + t0 * (P // 16)
        nc.gpsimd.dma_gather(
            out_ap=msg[:, t0:t1, :],
            in_ap=tbl_lo,
            idxs_ap=idx_lo_s[:, c0:c0 + n_idx // 16],
            num_idxs=n_idx,
            num_idxs_reg=n_idx,
            elem_size=P,
            queue_num=qctr % NSWQ,
            single_packet=(n_idx <= 1024),
        )
        qctr += 1
    if K_HI > 0:
        n_hi = wb * K_HI * P
        nc.gpsimd.dma_gather(
            out_ap=msg_hi[:, hi_off:hi_off + wb * K_HI, :],
            in_ap=tbl_hi,
            idxs_ap=idx_hi_s[:, int(hi_col_base[w]):int(hi_col_base[w]) + n_hi // 16],
            num_idxs=n_hi,
            num_idxs_reg=n_hi,
            elem_size=P,
            queue_num=qctr % NSWQ,
            single_packet=(n_hi <= 1024),
        )
        qctr += 1
    return qctr


def _emit_pass(nc, pools, pl, table_dram, idx_lo_s, idx_hi_s,
               dst_s, ew_s, iota_s, ones_s, bias_s, flush_fn):
    abl = os.environ.get("KERNEL_ABL", "")
    const_pool = pools.get("const")
    K_LO, K_HI, K = pl["K_LO"], pl["K_HI"], pl["K"]
    windows = pl["windows"]
    msg_pool, omega_pool, psum_pool = pools["msg"], pools["omega"], pools["psum"]

    max_wb = max(wb for _, wb in windows)
    msg_c = omega_c = None
    if "nogather" in abl:
        msg_c = const_pool.tile([P, max_wb * K, P], BF16, tag="msgc")
        nc.vector.memset(msg_c[:], 0.25)
    if "noomega" in abl:
        omega_c = const_pool.tile([P, max_wb * K * P], BF16, tag="omegac")
        nc.vector.memset(omega_c[:], 0.125)
    qctr = 0
    for w, (b0, wb) in enumerate(windows):
        wtiles = wb * K
        nlo_tiles = wb * K_LO
        if msg_c is not None:
            msg = msg_c
        else:
            msg = msg_pool.tile([P, max_wb * K, P], BF16, tag="msg")
        hi_off = nlo_tiles
        omega = omega_c if omega_c is not None else omega_pool.tile(
            [P, max_wb * K * P], BF16, tag="omega")
        if "nogather" not in abl:
            qctr = _emit_gathers(nc, pl, w, b0, wb, table_dram, idx_lo_s,
                                 idx_hi_s, msg, msg, hi_off, qctr)
        gt0 = b0 * K
        for t in range(wtiles if "noomega" not in abl else 0):
            nc.vector.tensor_scalar(
                out=omega[:, t * P:(t + 1) * P],
                in0=iota_s,
                scalar1=dst_s[:, gt0 + t:gt0 + t + 1],
                scalar2=ew_s[:, gt0 + t:gt0 + t + 1],
                op0=mybir.AluOpType.is_equal,
                op1=mybir.AluOpType.mult,
            )
        for j in range(wb):
            brow = b0 + j
            acc = psum_pool.tile([P, P], F32, tag="acc", space="PSUM")
            for k in range(K_LO if "nomm" not in abl else 0):
                t = j * K_LO + k
                nc.tensor.matmul(
                    out=acc[:], lhsT=omega[:, t * P:(t + 1) * P],
                    rhs=msg[:, t, :], start=(k == 0), stop=False)
            for k in range(K_HI if "nomm" not in abl else 0):
                t = wb * K_LO + j * K_HI + k
                nc.tensor.matmul(
                    out=acc[:], lhsT=omega[:, t * P:(t + 1) * P],
                    rhs=msg[:, t, :], start=False, stop=False)
            nc.tensor.matmul(out=acc[:], lhsT=ones_s, rhs=bias_s,
                             start=("nomm" in abl), stop=True)
            flush_fn(brow, acc)


def _build_program(meta, HID, OUT):
    pl1, pl2 = meta["pass1"], meta["pass2"]
    nblk = meta["nblk"]
    CH = meta["CH"]
    ROWS1, PAD_CORE, PAD_N = meta["ROWS1"], meta["PAD_CORE"], meta["PAD_N"]
    NT1 = ROWS1 // P
    HOUT = 2 * OUT
    abl = os.environ.get("KERNEL_ABL", "")

    scratch = int(os.environ.get("KERNEL_SCRATCH", "16384"))
    nc = bacc.Bacc(num_swdge_queues=NSWQ, dynamic_dma_scratch_size=scratch)
    xT_t = nc.declare_dram_parameter("xT", [P, ROWS1], BF16, isOutput=False)
    W1_t = nc.declare_dram_parameter("W1", [P, HID], BF16, isOutput=False)
    Wcat_t = nc.declare_dram_parameter("Wcat", [HID, HOUT], BF16, isOutput=False)
    b1_t = nc.declare_dram_parameter("b1", [1, HID], BF16, isOutput=False)
    bcat_t = nc.declare_dram_parameter("bcat", [1, HOUT], BF16, isOutput=False)
    iota_t = nc.declare_dram_parameter("iota", [P, P], BF16, isOutput=False)

    lo1_t = nc.declare_dram_parameter("lo1", [P, pl1["lo_idx"].shape[2]], I16, isOutput=False)
    hi1_t = nc.declare_dram_parameter("hi1", [P, pl1["hi_idx"].shape[2]], I16, isOutput=False)
    lo2_t = nc.declare_dram_parameter("lo2", [P, pl2["lo_idx"].shape[2]], I16, isOutput=False)
    hi2_t = nc.declare_dram_parameter("hi2", [P, pl2["hi_idx"].shape[2]], I16, isOutput=False)
    dst1_t = nc.declare_dram_parameter("dst1", [P, pl1["TOT_TILES"]], F32, isOutput=False)
    ew1_t = nc.declare_dram_parameter("ew1", [P, pl1["TOT_TILES"]], F32, isOutput=False)
    dst2_t = nc.declare_dram_parameter("dst2", [P, pl2["TOT_TILES"]], F32, isOutput=False)
    ew2_t = nc.declare_dram_parameter("ew2", [P, pl2["TOT_TILES"]], F32, isOutput=False)

    mu_t = nc.declare_dram_parameter("mu", [PAD_CORE, OUT], F32, isOutput=True)
    ls_t = nc.declare_dram_parameter("ls", [PAD_CORE, OUT], F32, isOutput=True)

    table1 = nc.dram_tensor("table1", [ROWS1, HID], BF16)
    ag_in = nc.dram_tensor("ag_in", [PAD_CORE, HOUT], BF16)
    table2 = nc.dram_tensor("table2", [PAD_N, HOUT], BF16, addr_space="Shared")

    with tile.TileContext(nc) as tc, ExitStack() as ctx:
        const = ctx.enter_context(tc.tile_pool(name="const", bufs=1))
        xt_pool = ctx.enter_context(tc.tile_pool(name="xt", bufs=3))
        stage_pool = ctx.enter_context(tc.tile_pool(name="stage", bufs=3))
        msg_pool = ctx.enter_context(tc.tile_pool(name="msg", bufs=2))
        omega_pool = ctx.enter_context(tc.tile_pool(name="omega", bufs=2))
        psum_pool = ctx.enter_context(tc.tile_pool(name="psum", bufs=4, space="PSUM"))
        tp_pool = ctx.enter_context(tc.tile_pool(name="tpsum", bufs=2, space="PSUM"))

        def load_const(param, shape, dtype):
            s = const.tile(shape, dtype, tag=param.name)
            nc.sync.dma_start(out=s[:], in_=param[:])
            return s[:]

        W1_s = load_const(W1_t, [P, HID], BF16)
        Wcat_s = load_const(Wcat_t, [HID, HOUT], BF16)
        b1_s = load_const(b1_t, [1, HID], BF16)
        bcat_s = load_const(bcat_t, [1, HOUT], BF16)
        iota_s = load_const(iota_t, [P, P], BF16)
        lo1_s = load_const(lo1_t, [P, pl1["lo_idx"].shape[2]], I16)
        hi1_s = load_const(hi1_t, [P, pl1["hi_idx"].shape[2]], I16)
        lo2_s = load_const(lo2_t, [P, pl2["lo_idx"].shape[2]], I16)
        hi2_s = load_const(hi2_t, [P, pl2["hi_idx"].shape[2]], I16)
        dst1_s = load_const(dst1_t, [P, pl1["TOT_TILES"]], F32)
        ew1_s = load_const(ew1_t, [P, pl1["TOT_TILES"]], F32)
        dst2_s = load_const(dst2_t, [P, pl2["TOT_TILES"]], F32)
        ew2_s = load_const(ew2_t, [P, pl2["TOT_TILES"]], F32)

        ones_s = const.tile([1, P], BF16, tag="ones")
        nc.vector.memset(ones_s[:], 1.0)
        identity_s = const.tile([P, P], BF16, tag="identity")
        nc.vector.memset(identity_s[:], 0.0)
        nc.gpsimd.affine_select(
            out=identity_s[:], in_=identity_s[:],
            compare_op=mybir.AluOpType.not_equal, fill=1.0,
            base=0, pattern=[[-1, P]], channel_multiplier=1)



        h1T = const.tile([P, nblk * P], BF16, tag="h1T")

        # ---- Phase A: full x @ W1, batched slab writes of table1 ----
        XSLAB = int(os.environ.get("KERNEL_XSLAB", "8"))
        for nt0 in range(0, NT1, XSLAB):
            nsl = min(XSLAB, NT1 - nt0)
            xsl = xt_pool.tile([P, XSLAB * P], BF16, tag="xsl")
            nc.sync.dma_start(out=xsl[:, 0:nsl * P],
                              in_=xT_t[:, nt0 * P:(nt0 + nsl) * P])
            slab = stage_pool.tile([P, XSLAB * HID], BF16, tag="t1slab")
            for i in range(nsl):
                pr = psum_pool.tile([P, HID], F32, tag="acc", space="PSUM")
                nc.tensor.matmul(out=pr[:], lhsT=xsl[:, i * P:(i + 1) * P],
                                 rhs=W1_s, start=True, stop=True)
                if i % 2 == 0:
                    nc.scalar.copy(out=slab[:, i * HID:(i + 1) * HID], in_=pr[:])
                else:
                    nc.vector.tensor_scalar(
                        out=slab[:, i * HID:(i + 1) * HID], in0=pr[:],
                        scalar1=1.0, scalar2=None, op0=mybir.AluOpType.mult)
            nc.sync.dma_start(
                out=table1[nt0 * P:(nt0 + nsl) * P, :].rearrange(
                    "(i p) f -> p i f", p=P),
                in_=slab[:, 0:nsl * HID].rearrange("p (i f) -> p i f", f=HID))

        tc.strict_bb_all_engine_barrier()

        # ---- Phase B (+ fused C and chunked AllGather) ----
        pools = dict(msg=msg_pool, omega=omega_pool, psum=psum_pool, const=const)

        ag_state = dict(slab=None, c0=0)

        def flush_ag_chunk(c0, c1):
            ch = c1 - c0
            slab = ag_state["slab"]
            nc.sync.dma_start(
                out=ag_in[c0 * P:c1 * P, :].rearrange("(i p) f -> p i f", p=P),
                in_=slab[:, 0:ch * HOUT].rearrange("p (i f) -> p i f", f=HOUT))
            if "noAG" not in abl:
                base8 = NCORES * c0 * P
                nc.gpsimd.collective_compute(
                    "AllGather", mybir.AluOpType.bypass,
                    replica_groups=[list(range(NCORES))],
                    ins=[ag_in[c0 * P:c1 * P, :]],
                    outs=[table2[base8:base8 + NCORES * ch * P, :]])

        def flush1(brow, acc):
            h1tile = stage_pool.tile([P, HID], BF16, tag="h1tile")
            nc.scalar.activation(out=h1tile[:], in_=acc[:],
                                 func=mybir.ActivationFunctionType.Relu)
            tp = tp_pool.tile([P, P], BF16, tag="tp", space="PSUM")
            nc.tensor.transpose(out=tp[:], in_=h1tile[:], identity=identity_s)
            nc.scalar.copy(out=h1T[:, brow * P:(brow + 1) * P], in_=tp[:])
            # fused phase C: project this block and stage bf16 rows
            if brow % CH == 0:
                agslab = stage_pool.tile([P, CH * HOUT], BF16, tag="agslab")
                ag_state["slab"] = agslab
                ag_state["c0"] = brow
            pr = psum_pool.tile([P, HOUT], F32, tag="acc", space="PSUM")
            nc.tensor.matmul(out=pr[:], lhsT=h1T[:, brow * P:(brow + 1) * P],
                             rhs=Wcat_s, start=True, stop=True)
            o = brow - ag_state["c0"]
            nc.vector.tensor_scalar(
                out=ag_state["slab"][:, o * HOUT:(o + 1) * HOUT], in0=pr[:],
                scalar1=1.0, scalar2=None, op0=mybir.AluOpType.mult)
            if brow == nblk - 1 or brow % CH == CH - 1:
                flush_ag_chunk(ag_state["c0"], brow + 1)

        if "noB" not in abl:
            _emit_pass(nc, pools, pl1, table1, lo1_s, hi1_s,
                       dst1_s, ew1_s, iota_s, ones_s[:], b1_s, flush1)
        else:
            for c in range(0, nblk, CH):
                agslab = stage_pool.tile([P, CH * HOUT], BF16, tag="agslab")
                ag_state["slab"] = agslab
                ag_state["c0"] = c
                nc.vector.memset(ag_state["slab"][:], 0.05)
                flush_ag_chunk(c, min(nblk, c + CH))

        tc.strict_bb_all_engine_barrier()

        # ---- Phase D ----
        OCH = 7
        o_state = dict(slab=None, c0=0)

        def flush_out_chunk(c0, c1):
            ch = c1 - c0
            slab = o_state["slab"]
            sl = slab[:, 0:ch * HOUT].rearrange("p (i f) -> p i f", f=HOUT)
            nc.sync.dma_start(
                out=mu_t[c0 * P:c1 * P, :].rearrange("(i p) f -> p i f", p=P),
                in_=sl[:, :, 0:OUT])
            nc.sync.dma_start(
                out=ls_t[c0 * P:c1 * P, :].rearrange("(i p) f -> p i f", p=P),
                in_=sl[:, :, OUT:HOUT])

        def flush2(brow, acc):
            if brow % OCH == 0:
                oslab = stage_pool.tile([P, OCH * HOUT], F32, tag="oslab")
                o_state["slab"] = oslab
                o_state["c0"] = brow
            o = brow - o_state["c0"]
            if brow % 2 == 0:
                nc.scalar.copy(out=o_state["slab"][:, o * HOUT:(o + 1) * HOUT],
                               in_=acc[:])
            else:
                nc.vector.tensor_scalar(
                    out=o_state["slab"][:, o * HOUT:(o + 1) * HOUT], in0=acc[:],
                    scalar1=1.0, scalar2=None, op0=mybir.AluOpType.mult)
            if brow == nblk - 1 or brow % OCH == OCH - 1:
                flush_out_chunk(o_state["c0"], brow + 1)

        if "noD" not in abl:
            _emit_pass(nc, pools, pl2, table2, lo2_s, hi2_s,
                       dst2_s, ew2_s, iota_s, ones_s[:], bcat_s, flush2)

    nc.finalize()
    return nc


# ----------------------------------------------------------------------------
# Public entry
# ----------------------------------------------------------------------------

def _prepare(x, edge_index, weight, W1, b1, Wmu, bmu, Wls, bls):
    x = np.asarray(x)
    N, IN_CH = x.shape
    HID = np.asarray(W1).shape[1]
    OUT = np.asarray(Wmu).shape[1]
    meta = _preprocess(x, np.asarray(edge_index), np.asarray(weight))
    pl1, pl2 = meta["pass1"], meta["pass2"]

    nc = _build_program(meta, HID, OUT)

    xT = np.zeros((P, meta["ROWS1"]), np.float32)
    xT[:IN_CH, :N] = np.asarray(x, np.float32).T
    Wcat = np.concatenate([np.asarray(Wmu), np.asarray(Wls)], axis=1)
    bcat = np.concatenate([np.asarray(bmu), np.asarray(bls)])
    iota = np.tile(np.arange(P, dtype=np.float32)[None, :], (P, 1))

    common = {
        "xT": xT.astype(NPBF16),
        "W1": np.asarray(W1, np.float32).astype(NPBF16),
        "Wcat": Wcat.astype(np.float32).astype(NPBF16),
        "b1": np.asarray(b1, np.float32).astype(NPBF16)[None, :],
        "bcat": bcat.astype(np.float32).astype(NPBF16)[None, :],
        "iota": iota.astype(NPBF16),
    }
    in_maps = []
    for c in range(NCORES):
        m = dict(common)
        m["lo1"] = pl1["lo_idx"][c]
        m["hi1"] = pl1["hi_idx"][c]
        m["lo2"] = pl2["lo_idx"][c]
        m["hi2"] = pl2["hi_idx"][c]
        m["dst1"] = pl1["dst_slab"][c]
        m["ew1"] = pl1["ew_slab"][c]
        m["dst2"] = pl2["dst_slab"][c]
        m["ew2"] = pl2["ew_slab"][c]
        in_maps.append(m)
    return nc, in_maps, meta


def _postprocess(results, meta):
    mu_cat = np.concatenate([results[c]["mu"] for c in range(NCORES)])
    ls_cat = np.concatenate([results[c]["ls"] for c in range(NCORES)])
    mu = mu_cat[meta["permpos"]].astype(np.float32)
    ls = ls_cat[meta["permpos"]].astype(np.float32)
    return mu, ls


def _run(x, edge_index, weight, W1, b1, Wmu, bmu, Wls, bls, trace=False):
    nc, in_maps, meta = _prepare(x, edge_index, weight, W1, b1, Wmu, bmu, Wls, bls)
    res = run_bass_kernel_spmd(nc, in_maps, list(range(NCORES)), trace=trace)
    return _postprocess(res.results, meta), res


def kernel(x, edge_index, weight, W1, b1, Wmu, bmu, Wls, bls):
    (mu, ls), _ = _run(x, edge_index, weight, W1, b1, Wmu, bmu, Wls, bls)
    return mu, ls
